# revision 19
# baseline (speedup 1.0000x reference)
"""BinLinear (BatchNorm -> sign-binarize -> scaled binary linear) on 8 TRN2
NeuronCores.

Reference computation (fp32, full batch):
    mean/var over batch axis of x [8192, 4096]
    h  = (x - mean) * rsqrt(var + eps) * gamma + beta          [8192, 4096]
    hb = sign(h)
    out = (hb @ W.T) * alpha[None, :] * mean_abs(h, axis=1)    [8192, 4096]

Distribution: data-parallel over the batch (1024 rows per core); BN stats via
a 32 KB AllReduce (a tiny warmup AllReduce at t=0 absorbs the collective
bootstrap skew). W is pre-scaled by alpha and 2^16, transposed to [in, out]
on the host, and split by k: the first 16 k-tiles are bf16, the last 16 are
fp8 e4m3 run as true DoubleRow pairs (two k-slices per PE pass, 2x the bf16
matmul rate). The measured end-to-end rel err of this split on the reference
inputs is 1.889e-2 (fp8-only would be 2.65e-2, all-bf16 ~2e-3), deterministic
across runs.

The kernel returns the raw accumulator (bf16) and the per-row sum of |h|;
the per-row beta scale, the 2^-16 descale and the fp32 cast happen on the
host while gathering shards.

PE timeline per core: 64 output tiles x (16 bf16 + 8 DoubleRow) matmuls
= 786k PE cycles (vs 1049k all-bf16), with stationary loads hidden behind
the moving stream (verified by microbenchmark).
"""

import numpy as np
import ml_dtypes

import concourse.bacc as bacc
import concourse.mybir as mybir
import concourse.tile as tile
from concourse.bass_utils import run_bass_kernel_spmd

dt = mybir.dt
AF = mybir.ActivationFunctionType
ALU = mybir.AluOpType
PM = mybir.MatmulPerfMode

N_CORES = 8
B, D = 8192, 4096          # batch, features (D_in == D_out == 4096)
BS = B // N_CORES          # 1024 batch rows per core
KT = D // 128              # 32 k-tiles (features / partitions)
KB = 16                    # k-tiles 0..KB-1 run in bf16
KJ = (KT - KB) // 2        # 8 DoubleRow pair groups for k-tiles KB..KT-1
EPS = 1e-5
WSCALE = 2.0 ** 16         # fp8 range scale, folded into both W halves

TRACE = False              # set by test.py for profiling runs
LAST_RESULT = None

_nc_cache = None


def _build():
    nc = bacc.Bacc("TRN2", target_bir_lowering=False, debug=False,
                   num_devices=N_CORES)
    xt_d = nc.dram_tensor("xt", [D, BS], dt.float32, kind="ExternalInput").ap()
    wb_d = nc.dram_tensor("wb", [KB * 128, D], dt.bfloat16,
                          kind="ExternalInput").ap()
    w8_d = nc.dram_tensor("w8", [KJ * 128, 2, D], dt.float8e4,
                          kind="ExternalInput").ap()
    gb_d = nc.dram_tensor("gb", [128, 2 * KT], dt.float32,
                          kind="ExternalInput").ap()
    out_d = nc.dram_tensor("out", [BS, D], dt.bfloat16,
                           kind="ExternalOutput").ap()
    bsum_d = nc.dram_tensor("bsum", [1, BS], dt.float32,
                            kind="ExternalOutput").ap()

    with tile.TileContext(nc) as tc:
        with (
            tc.tile_pool(name="const", bufs=1) as const,
            tc.tile_pool(name="xs", bufs=4) as xsp,
            tc.tile_pool(name="sq", bufs=2) as sqp,
            tc.tile_pool(name="x2", bufs=8) as x2p,
            tc.tile_pool(name="x3", bufs=3) as x3p,
            tc.tile_pool(name="habs", bufs=1) as habsp,
            tc.tile_pool(name="hb", bufs=1) as hbp,
            tc.tile_pool(name="hp", bufs=1) as hpp,
            tc.tile_pool(name="wq", bufs=32) as wqp,
            tc.tile_pool(name="yb", bufs=4) as ybp,
            tc.tile_pool(name="ps", bufs=6, space="PSUM") as psp,
            tc.tile_pool(name="psb", bufs=1, space="PSUM") as psbp,
            tc.tile_pool(name="dram", bufs=1, space="DRAM") as dram,
        ):
            # ---- warmup collective: absorbs bootstrap barrier + core skew
            # while phase 1 streams x. ----------------------------------------
            warm_sb = const.tile([128, 1], dt.float32)
            nc.vector.memset(warm_sb[:], 1.0)
            warm_in = dram.tile([128, 1], dt.float32, name="warm_in",
                                tag="warm_in")
            warm_out = dram.tile([128, 1], dt.float32, name="warm_out",
                                 tag="warm_out")
            nc.gpsimd.dma_start(warm_in[:], warm_sb[:])
            nc.gpsimd.collective_compute(
                "AllReduce", ALU.add,
                replica_groups=[list(range(N_CORES))],
                ins=[warm_in.opt()], outs=[warm_out.opt()],
            )

            # ---- constants -------------------------------------------------
            gb_t = const.tile([128, 2 * KT], dt.float32)
            nc.sync.dma_start(gb_t[:], gb_d[:])
            eps_t = const.tile([128, 1], dt.float32)
            nc.vector.memset(eps_t[:], EPS)
            ones8 = const.tile([128, 2, 32], dt.float8e4)
            nc.vector.memset(ones8[:], 1.0)

            # ---- phase 1: per-shard feature sums ---------------------------
            # Asymmetric split (24/8): the first AllReduce covers all bf16
            # k-tiles plus the first 4 DoubleRow pairs, so the PE can run
            # 20 of each pass's 24 matmuls while the second AllReduce is
            # still in flight. stat columns per half: [sum | sumsq].
            HKS = [24, KT - 24]
            HBASE = [0, 2 * HKS[0]]
            KBASE = [0, HKS[0]]
            stat = const.tile([128, 2 * KT], dt.float32)
            for t in range(KT):
                half = 0 if t < HKS[0] else 1
                idx = t - KBASE[half]
                xs = xsp.tile([128, BS], dt.float32, name="xs")
                nc.sync.dma_start(xs[:], xt_d[t * 128:(t + 1) * 128, :])
                c = HBASE[half] + idx
                nc.vector.reduce_sum(stat[:, c:c + 1], xs[:],
                                     axis=mybir.AxisListType.X)
                sq = sqp.tile([128, BS], dt.float32, name="sq")
                c = HBASE[half] + HKS[half] + idx
                nc.scalar.activation(sq[:], xs[:], AF.Square,
                                     accum_out=stat[:, c:c + 1])

            # ---- two pipelined AllReduces (one per feature half). Both
            # input DMAs are issued before either result is read back, so
            # AR1 starts the moment AR0 leaves the ring (a result read on
            # the same queue would stall AR1's input until AR0 completed).
            a_h = [const.tile([128, HKS[h]], dt.float32, name=f"a{h}",
                              tag=f"a{h}") for h in range(2)]
            b_h = [const.tile([128, HKS[h]], dt.float32, name=f"b{h}",
                              tag=f"b{h}") for h in range(2)]
            salls = []
            for half in range(2):
                HK = HKS[half]
                hs = slice(HBASE[half], HBASE[half] + 2 * HK)
                inb = dram.tile([128, 2 * HK], dt.float32, name=f"inb{half}",
                                tag=f"inb{half}")
                outb = dram.tile([128, 2 * HK], dt.float32,
                                 name=f"outb{half}", tag=f"outb{half}")
                nc.scalar.dma_start(inb[:], stat[:, hs])
                nc.gpsimd.collective_compute(
                    "AllReduce", ALU.add,
                    replica_groups=[list(range(N_CORES))],
                    ins=[inb.opt()], outs=[outb.opt()],
                )
                # Result read on the gpsimd queue, BETWEEN the two
                # collectives for half 0: the scalar queue must stay clear so
                # the Sign stream is not serialized behind an AllReduce wait.
                sall = const.tile([128, 2 * HK], dt.float32,
                                  name=f"sall{half}", tag=f"sall{half}")
                nc.gpsimd.dma_start(sall[:], outb[:])
                salls.append(sall)
            for half in range(2):
                HK = HKS[half]
                sall = salls[half]

                # BN coefficients: a = gamma/std, b = beta - mean*a
                mean = const.tile([128, HK], dt.float32, name=f"mean{half}",
                                  tag=f"mean{half}")
                nc.vector.tensor_scalar_mul(mean[:], sall[:, 0:HK], 1.0 / B)
                var = const.tile([128, HK], dt.float32, name=f"var{half}",
                                 tag=f"var{half}")
                nc.vector.tensor_scalar_mul(var[:], sall[:, HK:2 * HK],
                                            1.0 / B)
                msq = const.tile([128, HK], dt.float32, name=f"msq{half}",
                                 tag=f"msq{half}")
                nc.vector.tensor_mul(msq[:], mean[:], mean[:])
                nc.vector.tensor_sub(var[:], var[:], msq[:])
                std = const.tile([128, HK], dt.float32, name=f"std{half}",
                                 tag=f"std{half}")
                nc.scalar.activation(std[:], var[:], AF.Sqrt,
                                     bias=eps_t[:, 0:1], scale=1.0)
                ivs = const.tile([128, HK], dt.float32, name=f"ivs{half}",
                                 tag=f"ivs{half}")
                nc.vector.reciprocal(ivs[:], std[:])
                ks = slice(KBASE[half], KBASE[half] + HK)
                nc.vector.tensor_mul(a_h[half][:], ivs[:], gb_t[:, ks])
                nc.vector.tensor_mul(b_h[half][:], mean[:], a_h[half][:])
                nc.vector.tensor_sub(
                    b_h[half][:],
                    gb_t[:, KT + KBASE[half]:KT + KBASE[half] + HK],
                    b_h[half][:])

            # ---- phase 2: binarize. hb = Sign(x*a + b) fused on ACT from a
            # second fp32 pass over x (sync queue, behind phase 1's reads).
            # bf16-range tiles land in plain fp8 tiles; fp8-range tiles land
            # as DoubleRow pair planes. ---------------------------------------
            def coef(t):
                h = 0 if t < HKS[0] else 1
                i = t - KBASE[h]
                return a_h[h][:, i:i + 1], b_h[h][:, i:i + 1]

            # W-set loading: sets 0 and 1 are emitted on the sync queue
            # BEFORE the x2 stream so the first passes' weights are on chip
            # before the x2 pool's buffer rotation (paced by Sign consumption)
            # throttles the queue. Later sets follow behind x2.
            def load_wset(oq):
                wbs = []
                for k in range(KB):
                    wt = wqp.tile([128, 512], dt.bfloat16, name="wt",
                                  tag="wt")
                    nc.sync.dma_start(
                        wt[:],
                        wb_d[k * 128:(k + 1) * 128, oq * 512:(oq + 1) * 512])
                    wbs.append(wt)
                w8s = []
                for j in range(KJ):
                    w8 = wqp.tile([128, 2, 512], dt.float8e4, name="w8",
                                  tag="w8", bufs=16)
                    nc.sync.dma_start(
                        w8[:],
                        w8_d[j * 128:(j + 1) * 128, :,
                             oq * 512:(oq + 1) * 512])
                    w8s.append(w8)
                return wbs, w8s

            wsets = [load_wset(0), load_wset(1)]

            hb = []
            hp = []
            for j in range(KJ):
                hp.append(hpp.tile([128, 2, BS], dt.float8e4, name=f"hp{j}",
                                   tag=f"hp{j}"))
            for t in range(KT):
                x2 = x2p.tile([128, BS], dt.float32, name="x2")
                nc.sync.dma_start(x2[:], xt_d[t * 128:(t + 1) * 128, :])
                if t < KB:
                    hbt = hbp.tile([128, BS], dt.float8e4, name=f"hb{t}",
                                   tag=f"hb{t}")
                    dst = hbt[:]
                    hb.append(hbt)
                else:
                    j, plane = divmod(t - KB, 2)
                    dst = hp[j][:, plane, :]
                at, bt_ = coef(t)
                nc.scalar.activation(dst, x2[:], AF.Sign, bias=bt_, scale=at)

            # ---- phase 3: out[bt, oq] accumulates 16 bf16 + 8 DoubleRow
            # matmuls per [128, 512] tile. W streams once (oq-major). --------
            # beta: |h| = Abs(x*a + b) on ACT from a third x pass (off the
            # critical path), partition-reduced by ones-matmuls on the PE,
            # spliced into the middle of phase 3.
            hpairs = []

            def emit_habs():
                # Third x pass on the gpsimd queue: lands behind the first oq
                # groups' output writes, keeping this 16MB re-read clear of
                # the W prefetch window that feeds the PE. All 16 |h| pair
                # tiles stay resident so the Abs stream never throttles the
                # gpsimd queue (output writes share it).
                for m in range(KT // 2):
                    hpair = habsp.tile([128, 2, BS], dt.float8e4,
                                       name=f"habs{m}", tag=f"habs{m}")
                    for plane in range(2):
                        t = 2 * m + plane
                        x3 = x3p.tile([128, BS], dt.float32, name="x3")
                        nc.gpsimd.dma_start(x3[:],
                                            xt_d[t * 128:(t + 1) * 128, :])
                        at, bt_ = coef(t)
                        nc.scalar.activation(hpair[:, plane, :], x3[:],
                                             AF.Abs, bias=bt_, scale=at)
                    hpairs.append(hpair)

            def emit_beta_mms():
                # |h| pairs in fp8 feed DoubleRow ones-matmuls (32 identical
                # output rows; row 0 is drained): half the PE cost of a bf16
                # ones-reduction. e4m3 |h| only perturbs beta by
                # ~3.6%/sqrt(4096).
                beta_ps = psbp.tile([32, BS], dt.float32, tag="beta", bufs=1)
                for m in range(KT // 2):
                    for half in range(BS // 512):
                        nc.tensor.matmul(
                            beta_ps[:, half * 512:(half + 1) * 512],
                            ones8[:],
                            hpairs[m][:, :, half * 512:(half + 1) * 512],
                            start=(m == 0), stop=(m == KT // 2 - 1),
                            perf_mode=PM.DoubleRow)
                bsum_sb = const.tile([1, BS], dt.float32)
                nc.vector.tensor_copy(bsum_sb[:], beta_ps[0:1, :])
                nc.scalar.dma_start(bsum_d[:], bsum_sb[:])

            for oq in range(D // 512):
                if oq >= 2:
                    wsets.append(load_wset(oq))
                wbs, w8s = wsets[oq]

                def emit_mms(pt, bt, js):
                    if js.start == 0:
                        for k in range(KB):
                            nc.tensor.matmul(
                                pt[:], hb[k][:, bt * 128:(bt + 1) * 128],
                                wbs[k][:], start=(k == 0), stop=False)
                    for j in range(js.start, js.stop):
                        nc.tensor.matmul(
                            pt[:], hp[j][:, :, bt * 128:(bt + 1) * 128],
                            w8s[j][:], start=False, stop=(j == KJ - 1),
                            perf_mode=PM.DoubleRow)

                def drain(pt, bt):
                    yb = ybp.tile([128, 512], dt.bfloat16, name="yb")
                    nc.vector.tensor_copy(yb[:], pt[:])
                    nc.gpsimd.dma_start(
                        out_d[bt * 128:(bt + 1) * 128,
                              oq * 512:(oq + 1) * 512], yb[:])

                if oq == 0:
                    # Bridge the second AllReduce: run the half-0-covered
                    # portion (16 bf16 + 4 DR) of four passes first, then
                    # finish them once the half-1 signs land.
                    pts = []
                    for bt in range(6):
                        pt = psp.tile([128, 512], dt.float32, name="pt",
                                      tag="pt")
                        emit_mms(pt, bt, slice(0, 4))
                        pts.append(pt)
                    for bt in range(6):
                        emit_mms(pts[bt], bt, slice(4, KJ))
                        drain(pts[bt], bt)
                    rest = range(6, BS // 128)
                else:
                    rest = range(BS // 128)
                for bt in rest:
                    pt = psp.tile([128, 512], dt.float32, name="pt", tag="pt")
                    emit_mms(pt, bt, slice(0, KJ))
                    drain(pt, bt)
                if oq == 2:
                    emit_habs()
                if oq == 5:
                    emit_beta_mms()

    nc.compile()
    return nc


def kernel(x, bn_gamma, bn_beta, W, alpha):
    global _nc_cache, LAST_RESULT
    x = np.ascontiguousarray(x, dtype=np.float32)
    W = np.asarray(W, dtype=np.float32)
    alpha = np.asarray(alpha, dtype=np.float32)

    # host prep: fold alpha and the fp8 range scale into W, transpose to
    # [in, out]; k-rows 0..2047 in bf16, 2048..4095 in fp8 e4m3 arranged as
    # DoubleRow pair planes (k-tile 16+2j -> plane 0, 17+2j -> plane 1).
    wt = np.ascontiguousarray((W * alpha[:, None]).T) * np.float32(WSCALE)
    wb = wt[:KB * 128].astype(ml_dtypes.bfloat16)
    w8flat = wt[KB * 128:].astype(ml_dtypes.float8_e4m3)
    w8 = np.ascontiguousarray(
        w8flat.reshape(KJ, 2, 128, D).swapaxes(1, 2).reshape(KJ * 128, 2, D))
    # gamma/beta in per-partition layout: gb[p, t] = gamma[t*128 + p]
    gb = np.concatenate(
        [np.asarray(bn_gamma, np.float32).reshape(KT, 128).T,
         np.asarray(bn_beta, np.float32).reshape(KT, 128).T], axis=1)
    gb = np.ascontiguousarray(gb)

    if _nc_cache is None:
        _nc_cache = _build()
    nc = _nc_cache

    in_maps = []
    for c in range(N_CORES):
        xT = np.ascontiguousarray(x[c * BS:(c + 1) * BS, :].T)
        in_maps.append({"xt": xT, "wb": wb, "w8": w8, "gb": gb})

    res = run_bass_kernel_spmd(nc, in_maps, core_ids=list(range(N_CORES)),
                               trace=TRACE)
    LAST_RESULT = res
    outs = []
    for c in range(N_CORES):
        raw = res.results[c]["out"].astype(np.float32)        # [BS, D]
        bsum = np.asarray(res.results[c]["bsum"], np.float32)  # [1, BS]
        scale = bsum[0] / np.float32(D * WSCALE)               # [BS]
        outs.append(raw * scale[:, None])
    return np.concatenate(outs, axis=0)


# revision 20
# speedup vs baseline: 1.0124x; 1.0124x over previous
"""BinLinear (BatchNorm -> sign-binarize -> scaled binary linear) on 8 TRN2
NeuronCores.

Reference computation (fp32, full batch):
    mean/var over batch axis of x [8192, 4096]
    h  = (x - mean) * rsqrt(var + eps) * gamma + beta          [8192, 4096]
    hb = sign(h)
    out = (hb @ W.T) * alpha[None, :] * mean_abs(h, axis=1)    [8192, 4096]

Distribution: data-parallel over the batch (1024 rows per core); BN stats via
a 32 KB AllReduce (a tiny warmup AllReduce at t=0 absorbs the collective
bootstrap skew). W is pre-scaled by alpha and 2^16, transposed to [in, out]
on the host, and split by k: the first 16 k-tiles are bf16, the last 16 are
fp8 e4m3 run as true DoubleRow pairs (two k-slices per PE pass, 2x the bf16
matmul rate). The measured end-to-end rel err of this split on the reference
inputs is 1.889e-2 (fp8-only would be 2.65e-2, all-bf16 ~2e-3), deterministic
across runs.

The kernel returns the raw accumulator (bf16) and the per-row sum of |h|;
the per-row beta scale, the 2^-16 descale and the fp32 cast happen on the
host while gathering shards.

PE timeline per core: 64 output tiles x (16 bf16 + 8 DoubleRow) matmuls
= 786k PE cycles (vs 1049k all-bf16), with stationary loads hidden behind
the moving stream (verified by microbenchmark).
"""

import numpy as np
import ml_dtypes

import concourse.bacc as bacc
import concourse.mybir as mybir
import concourse.tile as tile
from concourse.bass_utils import run_bass_kernel_spmd

dt = mybir.dt
AF = mybir.ActivationFunctionType
ALU = mybir.AluOpType
PM = mybir.MatmulPerfMode

N_CORES = 8
B, D = 8192, 4096          # batch, features (D_in == D_out == 4096)
BS = B // N_CORES          # 1024 batch rows per core
KT = D // 128              # 32 k-tiles (features / partitions)
KB = 16                    # k-tiles 0..KB-1 run in bf16
KJ = (KT - KB) // 2        # 8 DoubleRow pair groups for k-tiles KB..KT-1
EPS = 1e-5
WSCALE = 2.0 ** 16         # fp8 range scale, folded into both W halves

TRACE = False              # set by test.py for profiling runs
LAST_RESULT = None

_nc_cache = None


def _build():
    nc = bacc.Bacc("TRN2", target_bir_lowering=False, debug=False,
                   num_devices=N_CORES)
    xt_d = nc.dram_tensor("xt", [D, BS], dt.float32, kind="ExternalInput").ap()
    wb_d = nc.dram_tensor("wb", [KB * 128, D], dt.bfloat16,
                          kind="ExternalInput").ap()
    w8_d = nc.dram_tensor("w8", [KJ * 128, 2, D], dt.float8e4,
                          kind="ExternalInput").ap()
    gb_d = nc.dram_tensor("gb", [128, 2 * KT], dt.float32,
                          kind="ExternalInput").ap()
    out_d = nc.dram_tensor("out", [BS, D], dt.bfloat16,
                           kind="ExternalOutput").ap()
    bsum_d = nc.dram_tensor("bsum", [1, BS], dt.float32,
                            kind="ExternalOutput").ap()

    with tile.TileContext(nc) as tc:
        with (
            tc.tile_pool(name="const", bufs=1) as const,
            tc.tile_pool(name="xs", bufs=4) as xsp,
            tc.tile_pool(name="sq", bufs=2) as sqp,
            tc.tile_pool(name="x2", bufs=8) as x2p,
            tc.tile_pool(name="x3", bufs=3) as x3p,
            tc.tile_pool(name="habs", bufs=1) as habsp,
            tc.tile_pool(name="hb", bufs=1) as hbp,
            tc.tile_pool(name="hp", bufs=1) as hpp,
            tc.tile_pool(name="wq", bufs=32) as wqp,
            tc.tile_pool(name="yb", bufs=4) as ybp,
            tc.tile_pool(name="ps", bufs=6, space="PSUM") as psp,
            tc.tile_pool(name="psb", bufs=1, space="PSUM") as psbp,
            tc.tile_pool(name="dram", bufs=1, space="DRAM") as dram,
        ):
            # ---- warmup collective: absorbs bootstrap barrier + core skew
            # while phase 1 streams x. ----------------------------------------
            warm_sb = const.tile([128, 1], dt.float32)
            nc.vector.memset(warm_sb[:], 1.0)
            warm_in = dram.tile([128, 1], dt.float32, name="warm_in",
                                tag="warm_in")
            warm_out = dram.tile([128, 1], dt.float32, name="warm_out",
                                 tag="warm_out")
            nc.gpsimd.dma_start(warm_in[:], warm_sb[:])
            nc.gpsimd.collective_compute(
                "AllReduce", ALU.add,
                replica_groups=[list(range(N_CORES))],
                ins=[warm_in.opt()], outs=[warm_out.opt()],
            )

            # ---- constants -------------------------------------------------
            gb_t = const.tile([128, 2 * KT], dt.float32)
            nc.sync.dma_start(gb_t[:], gb_d[:])
            eps_t = const.tile([128, 1], dt.float32)
            nc.vector.memset(eps_t[:], EPS)
            ones8 = const.tile([128, 2, 32], dt.float8e4)
            nc.vector.memset(ones8[:], 1.0)

            # ---- phase 1: per-shard feature sums ---------------------------
            # Asymmetric split (24/8): the first AllReduce covers all bf16
            # k-tiles plus the first 4 DoubleRow pairs, so the PE can run
            # 20 of each pass's 24 matmuls while the second AllReduce is
            # still in flight. stat columns per half: [sum | sumsq].
            HKS = [24, KT - 24]
            HBASE = [0, 2 * HKS[0]]
            KBASE = [0, HKS[0]]
            stat = const.tile([128, 2 * KT], dt.float32)
            for t in range(KT):
                half = 0 if t < HKS[0] else 1
                idx = t - KBASE[half]
                xs = xsp.tile([128, BS], dt.float32, name="xs")
                nc.sync.dma_start(xs[:], xt_d[t * 128:(t + 1) * 128, :])
                c = HBASE[half] + idx
                nc.vector.reduce_sum(stat[:, c:c + 1], xs[:],
                                     axis=mybir.AxisListType.X)
                sq = sqp.tile([128, BS], dt.float32, name="sq")
                c = HBASE[half] + HKS[half] + idx
                nc.scalar.activation(sq[:], xs[:], AF.Square,
                                     accum_out=stat[:, c:c + 1])

            # ---- two pipelined AllReduces (one per feature half). Both
            # input DMAs are issued before either result is read back, so
            # AR1 starts the moment AR0 leaves the ring (a result read on
            # the same queue would stall AR1's input until AR0 completed).
            a_h = [const.tile([128, HKS[h]], dt.float32, name=f"a{h}",
                              tag=f"a{h}") for h in range(2)]
            b_h = [const.tile([128, HKS[h]], dt.float32, name=f"b{h}",
                              tag=f"b{h}") for h in range(2)]
            salls = []
            for half in range(2):
                HK = HKS[half]
                hs = slice(HBASE[half], HBASE[half] + 2 * HK)
                inb = dram.tile([128, 2 * HK], dt.float32, name=f"inb{half}",
                                tag=f"inb{half}")
                outb = dram.tile([128, 2 * HK], dt.float32,
                                 name=f"outb{half}", tag=f"outb{half}")
                nc.scalar.dma_start(inb[:], stat[:, hs])
                nc.gpsimd.collective_compute(
                    "AllReduce", ALU.add,
                    replica_groups=[list(range(N_CORES))],
                    ins=[inb.opt()], outs=[outb.opt()],
                )
                # Result read on the gpsimd queue, BETWEEN the two
                # collectives for half 0: the scalar queue must stay clear so
                # the Sign stream is not serialized behind an AllReduce wait.
                sall = const.tile([128, 2 * HK], dt.float32,
                                  name=f"sall{half}", tag=f"sall{half}")
                nc.gpsimd.dma_start(sall[:], outb[:])
                salls.append(sall)
            def emit_coeffs(half):
                HK = HKS[half]
                sall = salls[half]

                # BN coefficients: a = gamma/std, b = beta - mean*a
                mean = const.tile([128, HK], dt.float32, name=f"mean{half}",
                                  tag=f"mean{half}")
                nc.vector.tensor_scalar_mul(mean[:], sall[:, 0:HK], 1.0 / B)
                var = const.tile([128, HK], dt.float32, name=f"var{half}",
                                 tag=f"var{half}")
                nc.vector.tensor_scalar_mul(var[:], sall[:, HK:2 * HK],
                                            1.0 / B)
                msq = const.tile([128, HK], dt.float32, name=f"msq{half}",
                                 tag=f"msq{half}")
                nc.vector.tensor_mul(msq[:], mean[:], mean[:])
                nc.vector.tensor_sub(var[:], var[:], msq[:])
                std = const.tile([128, HK], dt.float32, name=f"std{half}",
                                 tag=f"std{half}")
                nc.scalar.activation(std[:], var[:], AF.Sqrt,
                                     bias=eps_t[:, 0:1], scale=1.0)
                ivs = const.tile([128, HK], dt.float32, name=f"ivs{half}",
                                 tag=f"ivs{half}")
                nc.vector.reciprocal(ivs[:], std[:])
                ks = slice(KBASE[half], KBASE[half] + HK)
                nc.vector.tensor_mul(a_h[half][:], ivs[:], gb_t[:, ks])
                nc.vector.tensor_mul(b_h[half][:], mean[:], a_h[half][:])
                nc.vector.tensor_sub(
                    b_h[half][:],
                    gb_t[:, KT + KBASE[half]:KT + KBASE[half] + HK],
                    b_h[half][:])

            # ---- phase 2: binarize. hb = Sign(x*a + b) fused on ACT from a
            # second fp32 pass over x (sync queue, behind phase 1's reads).
            # bf16-range tiles land in plain fp8 tiles; fp8-range tiles land
            # as DoubleRow pair planes. ---------------------------------------
            def coef(t):
                h = 0 if t < HKS[0] else 1
                i = t - KBASE[h]
                return a_h[h][:, i:i + 1], b_h[h][:, i:i + 1]

            # W-set loading: sets 0 and 1 are emitted on the sync queue
            # BEFORE the x2 stream so the first passes' weights are on chip
            # before the x2 pool's buffer rotation (paced by Sign consumption)
            # throttles the queue. Later sets follow behind x2.
            def load_wset(oq):
                wbs = []
                for k in range(KB):
                    wt = wqp.tile([128, 512], dt.bfloat16, name="wt",
                                  tag="wt")
                    nc.sync.dma_start(
                        wt[:],
                        wb_d[k * 128:(k + 1) * 128, oq * 512:(oq + 1) * 512])
                    wbs.append(wt)
                w8s = []
                for j in range(KJ):
                    w8 = wqp.tile([128, 2, 512], dt.float8e4, name="w8",
                                  tag="w8", bufs=16)
                    nc.sync.dma_start(
                        w8[:],
                        w8_d[j * 128:(j + 1) * 128, :,
                             oq * 512:(oq + 1) * 512])
                    w8s.append(w8)
                return wbs, w8s

            wsets = [load_wset(0), load_wset(1)]

            hb = []
            hp = []
            for j in range(KJ):
                hp.append(hpp.tile([128, 2, BS], dt.float8e4, name=f"hp{j}",
                                   tag=f"hp{j}"))

            def emit_sign(t):
                x2 = x2p.tile([128, BS], dt.float32, name="x2")
                nc.sync.dma_start(x2[:], xt_d[t * 128:(t + 1) * 128, :])
                if t < KB:
                    hbt = hbp.tile([128, BS], dt.float8e4, name=f"hb{t}",
                                   tag=f"hb{t}")
                    dst = hbt[:]
                    hb.append(hbt)
                else:
                    j, plane = divmod(t - KB, 2)
                    dst = hp[j][:, plane, :]
                at, bt_ = coef(t)
                nc.scalar.activation(dst, x2[:], AF.Sign, bias=bt_, scale=at)

            # Interleave coefficient chains with the sign blocks: half 1's
            # Sqrt (an ACT op gated on the second AllReduce) must sit BEHIND
            # the half-0 signs on the in-order ACT queue, or it stalls them.
            emit_coeffs(0)
            for t in range(HKS[0]):
                emit_sign(t)
            emit_coeffs(1)
            for t in range(HKS[0], KT):
                emit_sign(t)

            # ---- phase 3: out[bt, oq] accumulates 16 bf16 + 8 DoubleRow
            # matmuls per [128, 512] tile. W streams once (oq-major). --------
            # beta: |h| = Abs(x*a + b) on ACT from a third x pass (off the
            # critical path), partition-reduced by ones-matmuls on the PE,
            # spliced into the middle of phase 3.
            hpairs = []

            def emit_habs():
                # Third x pass on the gpsimd queue: lands behind the first oq
                # groups' output writes, keeping this 16MB re-read clear of
                # the W prefetch window that feeds the PE. All 16 |h| pair
                # tiles stay resident so the Abs stream never throttles the
                # gpsimd queue (output writes share it).
                for m in range(KT // 2):
                    hpair = habsp.tile([128, 2, BS], dt.float8e4,
                                       name=f"habs{m}", tag=f"habs{m}")
                    for plane in range(2):
                        t = 2 * m + plane
                        x3 = x3p.tile([128, BS], dt.float32, name="x3")
                        nc.gpsimd.dma_start(x3[:],
                                            xt_d[t * 128:(t + 1) * 128, :])
                        at, bt_ = coef(t)
                        nc.scalar.activation(hpair[:, plane, :], x3[:],
                                             AF.Abs, bias=bt_, scale=at)
                    hpairs.append(hpair)

            def emit_beta_mms():
                # |h| pairs in fp8 feed DoubleRow ones-matmuls (32 identical
                # output rows; row 0 is drained): half the PE cost of a bf16
                # ones-reduction. e4m3 |h| only perturbs beta by
                # ~3.6%/sqrt(4096).
                beta_ps = psbp.tile([32, BS], dt.float32, tag="beta", bufs=1)
                for m in range(KT // 2):
                    for half in range(BS // 512):
                        nc.tensor.matmul(
                            beta_ps[:, half * 512:(half + 1) * 512],
                            ones8[:],
                            hpairs[m][:, :, half * 512:(half + 1) * 512],
                            start=(m == 0), stop=(m == KT // 2 - 1),
                            perf_mode=PM.DoubleRow)
                bsum_sb = const.tile([1, BS], dt.float32)
                nc.vector.tensor_copy(bsum_sb[:], beta_ps[0:1, :])
                nc.scalar.dma_start(bsum_d[:], bsum_sb[:])

            for oq in range(D // 512):
                if oq >= 2:
                    wsets.append(load_wset(oq))
                wbs, w8s = wsets[oq]

                def emit_mms(pt, bt, js):
                    if js.start == 0:
                        for k in range(KB):
                            nc.tensor.matmul(
                                pt[:], hb[k][:, bt * 128:(bt + 1) * 128],
                                wbs[k][:], start=(k == 0), stop=False)
                    for j in range(js.start, js.stop):
                        nc.tensor.matmul(
                            pt[:], hp[j][:, :, bt * 128:(bt + 1) * 128],
                            w8s[j][:], start=False, stop=(j == KJ - 1),
                            perf_mode=PM.DoubleRow)

                def drain(pt, bt):
                    yb = ybp.tile([128, 512], dt.bfloat16, name="yb")
                    nc.vector.tensor_copy(yb[:], pt[:])
                    nc.gpsimd.dma_start(
                        out_d[bt * 128:(bt + 1) * 128,
                              oq * 512:(oq + 1) * 512], yb[:])

                if oq == 0:
                    # Bridge the AllReduce window: six passes advance k-major
                    # (each new sign tile feeds six matmuls) through the
                    # half-0-covered range, then finish once the half-1 signs
                    # land.
                    pts = [psp.tile([128, 512], dt.float32, name="pt",
                                    tag="pt") for _ in range(6)]
                    for k in range(KB):
                        for bt in range(6):
                            nc.tensor.matmul(
                                pts[bt][:], hb[k][:, bt * 128:(bt + 1) * 128],
                                wbs[k][:], start=(k == 0), stop=False)
                    for j in range(4):
                        for bt in range(6):
                            nc.tensor.matmul(
                                pts[bt][:],
                                hp[j][:, :, bt * 128:(bt + 1) * 128],
                                w8s[j][:], start=False, stop=False,
                                perf_mode=PM.DoubleRow)
                    for bt in range(6):
                        emit_mms(pts[bt], bt, slice(4, KJ))
                        drain(pts[bt], bt)
                    rest = range(6, BS // 128)
                else:
                    rest = range(BS // 128)
                for bt in rest:
                    pt = psp.tile([128, 512], dt.float32, name="pt", tag="pt")
                    emit_mms(pt, bt, slice(0, KJ))
                    drain(pt, bt)
                if oq == 2:
                    emit_habs()
                if oq == 5:
                    emit_beta_mms()

    nc.compile()
    return nc


def kernel(x, bn_gamma, bn_beta, W, alpha):
    global _nc_cache, LAST_RESULT
    x = np.ascontiguousarray(x, dtype=np.float32)
    W = np.asarray(W, dtype=np.float32)
    alpha = np.asarray(alpha, dtype=np.float32)

    # host prep: fold alpha and the fp8 range scale into W, transpose to
    # [in, out]; k-rows 0..2047 in bf16, 2048..4095 in fp8 e4m3 arranged as
    # DoubleRow pair planes (k-tile 16+2j -> plane 0, 17+2j -> plane 1).
    wt = np.ascontiguousarray((W * alpha[:, None]).T) * np.float32(WSCALE)
    wb = wt[:KB * 128].astype(ml_dtypes.bfloat16)
    w8flat = wt[KB * 128:].astype(ml_dtypes.float8_e4m3)
    w8 = np.ascontiguousarray(
        w8flat.reshape(KJ, 2, 128, D).swapaxes(1, 2).reshape(KJ * 128, 2, D))
    # gamma/beta in per-partition layout: gb[p, t] = gamma[t*128 + p]
    gb = np.concatenate(
        [np.asarray(bn_gamma, np.float32).reshape(KT, 128).T,
         np.asarray(bn_beta, np.float32).reshape(KT, 128).T], axis=1)
    gb = np.ascontiguousarray(gb)

    if _nc_cache is None:
        _nc_cache = _build()
    nc = _nc_cache

    in_maps = []
    for c in range(N_CORES):
        xT = np.ascontiguousarray(x[c * BS:(c + 1) * BS, :].T)
        in_maps.append({"xt": xT, "wb": wb, "w8": w8, "gb": gb})

    res = run_bass_kernel_spmd(nc, in_maps, core_ids=list(range(N_CORES)),
                               trace=TRACE)
    LAST_RESULT = res
    outs = []
    for c in range(N_CORES):
        raw = res.results[c]["out"].astype(np.float32)        # [BS, D]
        bsum = np.asarray(res.results[c]["bsum"], np.float32)  # [1, BS]
        scale = bsum[0] / np.float32(D * WSCALE)               # [BS]
        outs.append(raw * scale[:, None])
    return np.concatenate(outs, axis=0)


# revision 23
# speedup vs baseline: 1.0259x; 1.0134x over previous
"""BinLinear (BatchNorm -> sign-binarize -> scaled binary linear) on 8 TRN2
NeuronCores.

Reference computation (fp32, full batch):
    mean/var over batch axis of x [8192, 4096]
    h  = (x - mean) * rsqrt(var + eps) * gamma + beta          [8192, 4096]
    hb = sign(h)
    out = (hb @ W.T) * alpha[None, :] * mean_abs(h, axis=1)    [8192, 4096]

Distribution: data-parallel over the batch (1024 rows per core); BN stats via
a 32 KB AllReduce (a tiny warmup AllReduce at t=0 absorbs the collective
bootstrap skew). W is pre-scaled by alpha and 2^16, transposed to [in, out]
on the host, and split by k: the first 16 k-tiles are bf16, the last 16 are
fp8 e4m3 run as true DoubleRow pairs (two k-slices per PE pass, 2x the bf16
matmul rate). The measured end-to-end rel err of this split on the reference
inputs is 1.889e-2 (fp8-only would be 2.65e-2, all-bf16 ~2e-3), deterministic
across runs.

The kernel returns the raw accumulator (bf16) and the per-row sum of |h|;
the per-row beta scale, the 2^-16 descale and the fp32 cast happen on the
host while gathering shards.

PE timeline per core: 64 output tiles x (16 bf16 + 8 DoubleRow) matmuls
= 786k PE cycles (vs 1049k all-bf16), with stationary loads hidden behind
the moving stream (verified by microbenchmark).
"""

import numpy as np
import ml_dtypes

import concourse.bacc as bacc
import concourse.mybir as mybir
import concourse.tile as tile
from concourse.tile import add_dep_helper
from concourse.bass_utils import run_bass_kernel_spmd

dt = mybir.dt
AF = mybir.ActivationFunctionType
ALU = mybir.AluOpType
PM = mybir.MatmulPerfMode

N_CORES = 8
B, D = 8192, 4096          # batch, features (D_in == D_out == 4096)
BS = B // N_CORES          # 1024 batch rows per core
KT = D // 128              # 32 k-tiles (features / partitions)
KB = 16                    # k-tiles 0..KB-1 run in bf16
KJ = (KT - KB) // 2        # 8 DoubleRow pair groups for k-tiles KB..KT-1
EPS = 1e-5
WSCALE = 2.0 ** 16         # fp8 range scale, folded into both W halves

TRACE = False              # set by test.py for profiling runs
LAST_RESULT = None

_nc_cache = None


def _build():
    nc = bacc.Bacc("TRN2", target_bir_lowering=False, debug=False,
                   num_devices=N_CORES)
    xt_d = nc.dram_tensor("xt", [D, BS], dt.float32, kind="ExternalInput").ap()
    wb_d = nc.dram_tensor("wb", [KB * 128, D], dt.bfloat16,
                          kind="ExternalInput").ap()
    w8_d = nc.dram_tensor("w8", [KJ * 128, 2, D], dt.float8e4,
                          kind="ExternalInput").ap()
    gb_d = nc.dram_tensor("gb", [128, 2 * KT], dt.float32,
                          kind="ExternalInput").ap()
    out_d = nc.dram_tensor("out", [BS, D], dt.bfloat16,
                           kind="ExternalOutput").ap()
    bsum_d = nc.dram_tensor("bsum", [1, BS], dt.float32,
                            kind="ExternalOutput").ap()

    with tile.TileContext(nc) as tc:
        with (
            tc.tile_pool(name="const", bufs=1) as const,
            tc.tile_pool(name="xs", bufs=4) as xsp,
            tc.tile_pool(name="sq", bufs=2) as sqp,
            tc.tile_pool(name="x2", bufs=8) as x2p,
            tc.tile_pool(name="x3", bufs=3) as x3p,
            tc.tile_pool(name="habs", bufs=1) as habsp,
            tc.tile_pool(name="hb", bufs=1) as hbp,
            tc.tile_pool(name="hp", bufs=1) as hpp,
            tc.tile_pool(name="wq", bufs=32) as wqp,
            tc.tile_pool(name="yb", bufs=4) as ybp,
            tc.tile_pool(name="ps", bufs=6, space="PSUM") as psp,
            tc.tile_pool(name="psb", bufs=1, space="PSUM") as psbp,
            tc.tile_pool(name="dram", bufs=1, space="DRAM") as dram,
        ):
            # ---- warmup collective: absorbs bootstrap barrier + core skew
            # while phase 1 streams x. ----------------------------------------
            warm_sb = const.tile([128, 1], dt.float32)
            nc.vector.memset(warm_sb[:], 1.0)
            warm_in = dram.tile([128, 1], dt.float32, name="warm_in",
                                tag="warm_in")
            warm_out = dram.tile([128, 1], dt.float32, name="warm_out",
                                 tag="warm_out")
            nc.gpsimd.dma_start(warm_in[:], warm_sb[:])
            nc.gpsimd.collective_compute(
                "AllReduce", ALU.add,
                replica_groups=[list(range(N_CORES))],
                ins=[warm_in.opt()], outs=[warm_out.opt()],
            )

            # ---- constants -------------------------------------------------
            gb_t = const.tile([128, 2 * KT], dt.float32)
            nc.sync.dma_start(gb_t[:], gb_d[:])
            eps_t = const.tile([128, 1], dt.float32)
            nc.vector.memset(eps_t[:], EPS)
            ones8 = const.tile([128, 2, 32], dt.float8e4)
            nc.vector.memset(ones8[:], 1.0)

            # ---- phase 1: per-shard feature sums ---------------------------
            # Asymmetric split (24/8): the first AllReduce covers all bf16
            # k-tiles plus the first 4 DoubleRow pairs, so the PE can run
            # 20 of each pass's 24 matmuls while the second AllReduce is
            # still in flight. stat columns per half: [sum | sumsq].
            HKS = [24, KT - 24]
            HBASE = [0, 2 * HKS[0]]
            KBASE = [0, HKS[0]]
            stat = const.tile([128, 2 * KT], dt.float32)
            for t in range(KT):
                half = 0 if t < HKS[0] else 1
                idx = t - KBASE[half]
                xs = xsp.tile([128, BS], dt.float32, name="xs")
                nc.sync.dma_start(xs[:], xt_d[t * 128:(t + 1) * 128, :])
                c = HBASE[half] + idx
                nc.vector.reduce_sum(stat[:, c:c + 1], xs[:],
                                     axis=mybir.AxisListType.X)
                sq = sqp.tile([128, BS], dt.float32, name="sq")
                c = HBASE[half] + HKS[half] + idx
                nc.scalar.activation(sq[:], xs[:], AF.Square,
                                     accum_out=stat[:, c:c + 1])

            # ---- two pipelined AllReduces (one per feature half). Both
            # input DMAs are issued before either result is read back, so
            # AR1 starts the moment AR0 leaves the ring (a result read on
            # the same queue would stall AR1's input until AR0 completed).
            a_h = [const.tile([128, HKS[h]], dt.float32, name=f"a{h}",
                              tag=f"a{h}") for h in range(2)]
            b_h = [const.tile([128, HKS[h]], dt.float32, name=f"b{h}",
                              tag=f"b{h}") for h in range(2)]
            salls = []
            for half in range(2):
                HK = HKS[half]
                hs = slice(HBASE[half], HBASE[half] + 2 * HK)
                inb = dram.tile([128, 2 * HK], dt.float32, name=f"inb{half}",
                                tag=f"inb{half}")
                outb = dram.tile([128, 2 * HK], dt.float32,
                                 name=f"outb{half}", tag=f"outb{half}")
                nc.scalar.dma_start(inb[:], stat[:, hs])
                nc.gpsimd.collective_compute(
                    "AllReduce", ALU.add,
                    replica_groups=[list(range(N_CORES))],
                    ins=[inb.opt()], outs=[outb.opt()],
                )
                # Result read on the gpsimd queue, BETWEEN the two
                # collectives for half 0: the scalar queue must stay clear so
                # the Sign stream is not serialized behind an AllReduce wait.
                sall = const.tile([128, 2 * HK], dt.float32,
                                  name=f"sall{half}", tag=f"sall{half}")
                nc.gpsimd.dma_start(sall[:], outb[:])
                salls.append(sall)
            def emit_coeffs(half):
                HK = HKS[half]
                sall = salls[half]

                # BN coefficients: a = gamma/std, b = beta - mean*a
                mean = const.tile([128, HK], dt.float32, name=f"mean{half}",
                                  tag=f"mean{half}")
                nc.vector.tensor_scalar_mul(mean[:], sall[:, 0:HK], 1.0 / B)
                var = const.tile([128, HK], dt.float32, name=f"var{half}",
                                 tag=f"var{half}")
                nc.vector.tensor_scalar_mul(var[:], sall[:, HK:2 * HK],
                                            1.0 / B)
                msq = const.tile([128, HK], dt.float32, name=f"msq{half}",
                                 tag=f"msq{half}")
                nc.vector.tensor_mul(msq[:], mean[:], mean[:])
                nc.vector.tensor_sub(var[:], var[:], msq[:])
                std = const.tile([128, HK], dt.float32, name=f"std{half}",
                                 tag=f"std{half}")
                sqrt_inst = nc.scalar.activation(std[:], var[:], AF.Sqrt,
                                                 bias=eps_t[:, 0:1], scale=1.0)
                ivs = const.tile([128, HK], dt.float32, name=f"ivs{half}",
                                 tag=f"ivs{half}")
                nc.vector.reciprocal(ivs[:], std[:])
                ks = slice(KBASE[half], KBASE[half] + HK)
                nc.vector.tensor_mul(a_h[half][:], ivs[:], gb_t[:, ks])
                nc.vector.tensor_mul(b_h[half][:], mean[:], a_h[half][:])
                nc.vector.tensor_sub(
                    b_h[half][:],
                    gb_t[:, KT + KBASE[half]:KT + KBASE[half] + HK],
                    b_h[half][:])
                return sqrt_inst

            # ---- phase 2: binarize. hb = Sign(x*a + b) fused on ACT from a
            # second fp32 pass over x (sync queue, behind phase 1's reads).
            # bf16-range tiles land in plain fp8 tiles; fp8-range tiles land
            # as DoubleRow pair planes. ---------------------------------------
            def coef(t):
                h = 0 if t < HKS[0] else 1
                i = t - KBASE[h]
                return a_h[h][:, i:i + 1], b_h[h][:, i:i + 1]

            # The tile scheduler's timing model misjudges when AllReduce- and
            # late-DMA-gated ACT ops become ready and hoists them between the
            # Sign stream, where their semaphore waits would stall the
            # in-order ACT queue. Chain every ACT op explicitly in program
            # order instead.
            act_chain = [None]

            def chain_act(inst):
                if act_chain[0] is not None:
                    add_dep_helper(inst.ins, act_chain[0].ins, sync=True,
                                   reason="ACT program order")
                act_chain[0] = inst
                return inst

            # W-set loading: sets 0 and 1 are emitted on the sync queue
            # BEFORE the x2 stream so the first passes' weights are on chip
            # before the x2 pool's buffer rotation (paced by Sign consumption)
            # throttles the queue. Later sets follow behind x2.
            def load_wset(oq):
                wbs = []
                for k in range(KB):
                    wt = wqp.tile([128, 512], dt.bfloat16, name="wt",
                                  tag="wt")
                    nc.sync.dma_start(
                        wt[:],
                        wb_d[k * 128:(k + 1) * 128, oq * 512:(oq + 1) * 512])
                    wbs.append(wt)
                w8s = []
                for j in range(KJ):
                    w8 = wqp.tile([128, 2, 512], dt.float8e4, name="w8",
                                  tag="w8", bufs=16)
                    nc.sync.dma_start(
                        w8[:],
                        w8_d[j * 128:(j + 1) * 128, :,
                             oq * 512:(oq + 1) * 512])
                    w8s.append(w8)
                return wbs, w8s

            wsets = [load_wset(0), load_wset(1)]

            hb = []
            hp = []
            for j in range(KJ):
                hp.append(hpp.tile([128, 2, BS], dt.float8e4, name=f"hp{j}",
                                   tag=f"hp{j}"))

            def emit_sign(t):
                x2 = x2p.tile([128, BS], dt.float32, name="x2")
                nc.sync.dma_start(x2[:], xt_d[t * 128:(t + 1) * 128, :])
                if t < KB:
                    hbt = hbp.tile([128, BS], dt.float8e4, name=f"hb{t}",
                                   tag=f"hb{t}")
                    dst = hbt[:]
                    hb.append(hbt)
                else:
                    j, plane = divmod(t - KB, 2)
                    dst = hp[j][:, plane, :]
                at, bt_ = coef(t)
                return chain_act(nc.scalar.activation(dst, x2[:], AF.Sign,
                                                      bias=bt_, scale=at))

            # Interleave coefficient chains with the sign blocks: half 1's
            # Sqrt (an ACT op gated on the second AllReduce) must execute
            # BEHIND the half-0 signs on the in-order ACT queue, or it
            # stalls them until the AllReduce lands. The scheduler hoists it
            # otherwise, so pin it with an explicit edge.
            emit_coeffs(0)
            for t in range(HKS[0]):
                emit_sign(t)
            chain_act(emit_coeffs(1))
            for t in range(HKS[0], KT):
                emit_sign(t)

            # ---- phase 3: out[bt, oq] accumulates 16 bf16 + 8 DoubleRow
            # matmuls per [128, 512] tile. W streams once (oq-major). --------
            # beta: |h| = Abs(x*a + b) on ACT from a third x pass (off the
            # critical path), partition-reduced by ones-matmuls on the PE,
            # spliced into the middle of phase 3.
            hpairs = []

            def emit_habs():
                # Third x pass on the gpsimd queue: lands behind the first oq
                # groups' output writes, keeping this 16MB re-read clear of
                # the W prefetch window that feeds the PE. All 16 |h| pair
                # tiles stay resident so the Abs stream never throttles the
                # gpsimd queue (output writes share it).
                for m in range(KT // 2):
                    hpair = habsp.tile([128, 2, BS], dt.float8e4,
                                       name=f"habs{m}", tag=f"habs{m}")
                    for plane in range(2):
                        t = 2 * m + plane
                        x3 = x3p.tile([128, BS], dt.float32, name="x3")
                        nc.gpsimd.dma_start(x3[:],
                                            xt_d[t * 128:(t + 1) * 128, :])
                        at, bt_ = coef(t)
                        chain_act(nc.scalar.activation(hpair[:, plane, :],
                                                       x3[:], AF.Abs,
                                                       bias=bt_, scale=at))
                    hpairs.append(hpair)

            def emit_beta_mms():
                # |h| pairs in fp8 feed DoubleRow ones-matmuls (32 identical
                # output rows; row 0 is drained): half the PE cost of a bf16
                # ones-reduction. e4m3 |h| only perturbs beta by
                # ~3.6%/sqrt(4096).
                beta_ps = psbp.tile([32, BS], dt.float32, tag="beta", bufs=1)
                for m in range(KT // 2):
                    for half in range(BS // 512):
                        nc.tensor.matmul(
                            beta_ps[:, half * 512:(half + 1) * 512],
                            ones8[:],
                            hpairs[m][:, :, half * 512:(half + 1) * 512],
                            start=(m == 0), stop=(m == KT // 2 - 1),
                            perf_mode=PM.DoubleRow)
                bsum_sb = const.tile([1, BS], dt.float32)
                nc.vector.tensor_copy(bsum_sb[:], beta_ps[0:1, :])
                nc.scalar.dma_start(bsum_d[:], bsum_sb[:])

            for oq in range(D // 512):
                if oq >= 2:
                    wsets.append(load_wset(oq))
                wbs, w8s = wsets[oq]

                def emit_mms(pt, bt, js):
                    if js.start == 0:
                        for k in range(KB):
                            nc.tensor.matmul(
                                pt[:], hb[k][:, bt * 128:(bt + 1) * 128],
                                wbs[k][:], start=(k == 0), stop=False)
                    for j in range(js.start, js.stop):
                        nc.tensor.matmul(
                            pt[:], hp[j][:, :, bt * 128:(bt + 1) * 128],
                            w8s[j][:], start=False, stop=(j == KJ - 1),
                            perf_mode=PM.DoubleRow)

                def drain(pt, bt):
                    yb = ybp.tile([128, 512], dt.bfloat16, name="yb")
                    nc.vector.tensor_copy(yb[:], pt[:])
                    nc.gpsimd.dma_start(
                        out_d[bt * 128:(bt + 1) * 128,
                              oq * 512:(oq + 1) * 512], yb[:])

                if oq == 0:
                    # Bridge the AllReduce window: six passes advance k-major
                    # (each new sign tile feeds six matmuls) through the
                    # half-0-covered range, then finish once the half-1 signs
                    # land.
                    pts = [psp.tile([128, 512], dt.float32, name="pt",
                                    tag="pt") for _ in range(6)]
                    for k in range(KB):
                        for bt in range(6):
                            nc.tensor.matmul(
                                pts[bt][:], hb[k][:, bt * 128:(bt + 1) * 128],
                                wbs[k][:], start=(k == 0), stop=False)
                    for j in range(4):
                        for bt in range(6):
                            nc.tensor.matmul(
                                pts[bt][:],
                                hp[j][:, :, bt * 128:(bt + 1) * 128],
                                w8s[j][:], start=False, stop=False,
                                perf_mode=PM.DoubleRow)
                    for bt in range(6):
                        emit_mms(pts[bt], bt, slice(4, KJ))
                        drain(pts[bt], bt)
                    rest = range(6, BS // 128)
                else:
                    rest = range(BS // 128)
                for bt in rest:
                    pt = psp.tile([128, 512], dt.float32, name="pt", tag="pt")
                    emit_mms(pt, bt, slice(0, KJ))
                    drain(pt, bt)
                if oq == 2:
                    emit_habs()
                if oq == 5:
                    emit_beta_mms()

    nc.compile()
    return nc


def kernel(x, bn_gamma, bn_beta, W, alpha):
    global _nc_cache, LAST_RESULT
    x = np.ascontiguousarray(x, dtype=np.float32)
    W = np.asarray(W, dtype=np.float32)
    alpha = np.asarray(alpha, dtype=np.float32)

    # host prep: fold alpha and the fp8 range scale into W, transpose to
    # [in, out]; k-rows 0..2047 in bf16, 2048..4095 in fp8 e4m3 arranged as
    # DoubleRow pair planes (k-tile 16+2j -> plane 0, 17+2j -> plane 1).
    wt = np.ascontiguousarray((W * alpha[:, None]).T) * np.float32(WSCALE)
    wb = wt[:KB * 128].astype(ml_dtypes.bfloat16)
    w8flat = wt[KB * 128:].astype(ml_dtypes.float8_e4m3)
    w8 = np.ascontiguousarray(
        w8flat.reshape(KJ, 2, 128, D).swapaxes(1, 2).reshape(KJ * 128, 2, D))
    # gamma/beta in per-partition layout: gb[p, t] = gamma[t*128 + p]
    gb = np.concatenate(
        [np.asarray(bn_gamma, np.float32).reshape(KT, 128).T,
         np.asarray(bn_beta, np.float32).reshape(KT, 128).T], axis=1)
    gb = np.ascontiguousarray(gb)

    if _nc_cache is None:
        _nc_cache = _build()
    nc = _nc_cache

    in_maps = []
    for c in range(N_CORES):
        xT = np.ascontiguousarray(x[c * BS:(c + 1) * BS, :].T)
        in_maps.append({"xt": xT, "wb": wb, "w8": w8, "gb": gb})

    res = run_bass_kernel_spmd(nc, in_maps, core_ids=list(range(N_CORES)),
                               trace=TRACE)
    LAST_RESULT = res
    outs = []
    for c in range(N_CORES):
        raw = res.results[c]["out"].astype(np.float32)        # [BS, D]
        bsum = np.asarray(res.results[c]["bsum"], np.float32)  # [1, BS]
        scale = bsum[0] / np.float32(D * WSCALE)               # [BS]
        outs.append(raw * scale[:, None])
    return np.concatenate(outs, axis=0)


# revision 24
# speedup vs baseline: 1.0301x; 1.0040x over previous
"""BinLinear (BatchNorm -> sign-binarize -> scaled binary linear) on 8 TRN2
NeuronCores.

Reference computation (fp32, full batch):
    mean/var over batch axis of x [8192, 4096]
    h  = (x - mean) * rsqrt(var + eps) * gamma + beta          [8192, 4096]
    hb = sign(h)
    out = (hb @ W.T) * alpha[None, :] * mean_abs(h, axis=1)    [8192, 4096]

Distribution: data-parallel over the batch (1024 rows per core); BN stats via
a 32 KB AllReduce (a tiny warmup AllReduce at t=0 absorbs the collective
bootstrap skew). W is pre-scaled by alpha and 2^16, transposed to [in, out]
on the host, and split by k: the first 16 k-tiles are bf16, the last 16 are
fp8 e4m3 run as true DoubleRow pairs (two k-slices per PE pass, 2x the bf16
matmul rate). The measured end-to-end rel err of this split on the reference
inputs is 1.889e-2 (fp8-only would be 2.65e-2, all-bf16 ~2e-3), deterministic
across runs.

The kernel returns the raw accumulator (bf16) and the per-row sum of |h|;
the per-row beta scale, the 2^-16 descale and the fp32 cast happen on the
host while gathering shards.

PE timeline per core: 64 output tiles x (16 bf16 + 8 DoubleRow) matmuls
= 786k PE cycles (vs 1049k all-bf16), with stationary loads hidden behind
the moving stream (verified by microbenchmark).
"""

import numpy as np
import ml_dtypes

import concourse.bacc as bacc
import concourse.mybir as mybir
import concourse.tile as tile
from concourse.tile import add_dep_helper
from concourse.bass_utils import run_bass_kernel_spmd

dt = mybir.dt
AF = mybir.ActivationFunctionType
ALU = mybir.AluOpType
PM = mybir.MatmulPerfMode

N_CORES = 8
B, D = 8192, 4096          # batch, features (D_in == D_out == 4096)
BS = B // N_CORES          # 1024 batch rows per core
KT = D // 128              # 32 k-tiles (features / partitions)
KB = 16                    # k-tiles 0..KB-1 run in bf16
KJ = (KT - KB) // 2        # 8 DoubleRow pair groups for k-tiles KB..KT-1
EPS = 1e-5
WSCALE = 2.0 ** 16         # fp8 range scale, folded into both W halves

TRACE = False              # set by test.py for profiling runs
LAST_RESULT = None

_nc_cache = None


def _build():
    nc = bacc.Bacc("TRN2", target_bir_lowering=False, debug=False,
                   num_devices=N_CORES)
    xt_d = nc.dram_tensor("xt", [D, BS], dt.float32, kind="ExternalInput").ap()
    wb_d = nc.dram_tensor("wb", [KB * 128, D], dt.bfloat16,
                          kind="ExternalInput").ap()
    w8_d = nc.dram_tensor("w8", [KJ * 128, 2, D], dt.float8e4,
                          kind="ExternalInput").ap()
    gb_d = nc.dram_tensor("gb", [128, 2 * KT], dt.float32,
                          kind="ExternalInput").ap()
    out_d = nc.dram_tensor("out", [BS, D], dt.bfloat16,
                           kind="ExternalOutput").ap()
    bsum_d = nc.dram_tensor("bsum", [1, BS], dt.float32,
                            kind="ExternalOutput").ap()

    with tile.TileContext(nc) as tc:
        with (
            tc.tile_pool(name="const", bufs=1) as const,
            tc.tile_pool(name="xs", bufs=4) as xsp,
            tc.tile_pool(name="sq", bufs=2) as sqp,
            tc.tile_pool(name="x2", bufs=8) as x2p,
            tc.tile_pool(name="x3", bufs=3) as x3p,
            tc.tile_pool(name="habs", bufs=1) as habsp,
            tc.tile_pool(name="hb", bufs=1) as hbp,
            tc.tile_pool(name="hp", bufs=1) as hpp,
            tc.tile_pool(name="wq", bufs=32) as wqp,
            tc.tile_pool(name="yb", bufs=4) as ybp,
            tc.tile_pool(name="ps", bufs=6, space="PSUM") as psp,
            tc.tile_pool(name="psb", bufs=1, space="PSUM") as psbp,
            tc.tile_pool(name="dram", bufs=1, space="DRAM") as dram,
        ):
            # ---- warmup collective: absorbs bootstrap barrier + core skew
            # while phase 1 streams x. ----------------------------------------
            warm_sb = const.tile([128, 1], dt.float32)
            nc.vector.memset(warm_sb[:], 1.0)
            warm_in = dram.tile([128, 1], dt.float32, name="warm_in",
                                tag="warm_in")
            warm_out = dram.tile([128, 1], dt.float32, name="warm_out",
                                 tag="warm_out")
            nc.gpsimd.dma_start(warm_in[:], warm_sb[:])
            nc.gpsimd.collective_compute(
                "AllReduce", ALU.add,
                replica_groups=[list(range(N_CORES))],
                ins=[warm_in.opt()], outs=[warm_out.opt()],
            )

            # ---- constants -------------------------------------------------
            # Dummy Sqrt as the FIRST activation: pins the initial ACT
            # function-table load to a set that contains square, sqrt, sign
            # AND abs, so no ~20us table reload lands mid-kernel between the
            # stats squares and the sign stream.
            warmact = const.tile([128, 1], dt.float32)
            nc.scalar.activation(warmact[:], warm_sb[:], AF.Sqrt)

            gb_t = const.tile([128, 2 * KT], dt.float32)
            nc.sync.dma_start(gb_t[:], gb_d[:])
            eps_t = const.tile([128, 1], dt.float32)
            nc.vector.memset(eps_t[:], EPS)
            ones8 = const.tile([128, 2, 32], dt.float8e4)
            nc.vector.memset(ones8[:], 1.0)

            # ---- phase 1: per-shard feature sums ---------------------------
            # Asymmetric split (24/8): the first AllReduce covers all bf16
            # k-tiles plus the first 4 DoubleRow pairs, so the PE can run
            # 20 of each pass's 24 matmuls while the second AllReduce is
            # still in flight. stat columns per half: [sum | sumsq].
            HKS = [24, KT - 24]
            HBASE = [0, 2 * HKS[0]]
            KBASE = [0, HKS[0]]
            stat = const.tile([128, 2 * KT], dt.float32)
            for t in range(KT):
                half = 0 if t < HKS[0] else 1
                idx = t - KBASE[half]
                xs = xsp.tile([128, BS], dt.float32, name="xs")
                nc.sync.dma_start(xs[:], xt_d[t * 128:(t + 1) * 128, :])
                c = HBASE[half] + idx
                nc.vector.reduce_sum(stat[:, c:c + 1], xs[:],
                                     axis=mybir.AxisListType.X)
                sq = sqp.tile([128, BS], dt.float32, name="sq")
                c = HBASE[half] + HKS[half] + idx
                nc.scalar.activation(sq[:], xs[:], AF.Square,
                                     accum_out=stat[:, c:c + 1])

            # ---- two pipelined AllReduces (one per feature half). Both
            # input DMAs are issued before either result is read back, so
            # AR1 starts the moment AR0 leaves the ring (a result read on
            # the same queue would stall AR1's input until AR0 completed).
            a_h = [const.tile([128, HKS[h]], dt.float32, name=f"a{h}",
                              tag=f"a{h}") for h in range(2)]
            b_h = [const.tile([128, HKS[h]], dt.float32, name=f"b{h}",
                              tag=f"b{h}") for h in range(2)]
            salls = []
            for half in range(2):
                HK = HKS[half]
                hs = slice(HBASE[half], HBASE[half] + 2 * HK)
                inb = dram.tile([128, 2 * HK], dt.float32, name=f"inb{half}",
                                tag=f"inb{half}")
                outb = dram.tile([128, 2 * HK], dt.float32,
                                 name=f"outb{half}", tag=f"outb{half}")
                nc.scalar.dma_start(inb[:], stat[:, hs])
                nc.gpsimd.collective_compute(
                    "AllReduce", ALU.add,
                    replica_groups=[list(range(N_CORES))],
                    ins=[inb.opt()], outs=[outb.opt()],
                )
                # Result read on the gpsimd queue, BETWEEN the two
                # collectives for half 0: the scalar queue must stay clear so
                # the Sign stream is not serialized behind an AllReduce wait.
                sall = const.tile([128, 2 * HK], dt.float32,
                                  name=f"sall{half}", tag=f"sall{half}")
                nc.gpsimd.dma_start(sall[:], outb[:])
                salls.append(sall)
            def emit_coeffs(half):
                HK = HKS[half]
                sall = salls[half]

                # BN coefficients: a = gamma/std, b = beta - mean*a
                mean = const.tile([128, HK], dt.float32, name=f"mean{half}",
                                  tag=f"mean{half}")
                nc.vector.tensor_scalar_mul(mean[:], sall[:, 0:HK], 1.0 / B)
                var = const.tile([128, HK], dt.float32, name=f"var{half}",
                                 tag=f"var{half}")
                nc.vector.tensor_scalar_mul(var[:], sall[:, HK:2 * HK],
                                            1.0 / B)
                msq = const.tile([128, HK], dt.float32, name=f"msq{half}",
                                 tag=f"msq{half}")
                nc.vector.tensor_mul(msq[:], mean[:], mean[:])
                nc.vector.tensor_sub(var[:], var[:], msq[:])
                std = const.tile([128, HK], dt.float32, name=f"std{half}",
                                 tag=f"std{half}")
                sqrt_inst = nc.scalar.activation(std[:], var[:], AF.Sqrt,
                                                 bias=eps_t[:, 0:1], scale=1.0)
                ivs = const.tile([128, HK], dt.float32, name=f"ivs{half}",
                                 tag=f"ivs{half}")
                nc.vector.reciprocal(ivs[:], std[:])
                ks = slice(KBASE[half], KBASE[half] + HK)
                nc.vector.tensor_mul(a_h[half][:], ivs[:], gb_t[:, ks])
                nc.vector.tensor_mul(b_h[half][:], mean[:], a_h[half][:])
                nc.vector.tensor_sub(
                    b_h[half][:],
                    gb_t[:, KT + KBASE[half]:KT + KBASE[half] + HK],
                    b_h[half][:])
                return sqrt_inst

            # ---- phase 2: binarize. hb = Sign(x*a + b) fused on ACT from a
            # second fp32 pass over x (sync queue, behind phase 1's reads).
            # bf16-range tiles land in plain fp8 tiles; fp8-range tiles land
            # as DoubleRow pair planes. ---------------------------------------
            def coef(t):
                h = 0 if t < HKS[0] else 1
                i = t - KBASE[h]
                return a_h[h][:, i:i + 1], b_h[h][:, i:i + 1]

            # The tile scheduler's timing model misjudges when AllReduce- and
            # late-DMA-gated ACT ops become ready and hoists them between the
            # Sign stream, where their semaphore waits would stall the
            # in-order ACT queue. Chain every ACT op explicitly in program
            # order instead.
            act_chain = [None]

            def chain_act(inst):
                if act_chain[0] is not None:
                    add_dep_helper(inst.ins, act_chain[0].ins, sync=True,
                                   reason="ACT program order")
                act_chain[0] = inst
                return inst

            # W-set loading: sets 0 and 1 are emitted on the sync queue
            # BEFORE the x2 stream so the first passes' weights are on chip
            # before the x2 pool's buffer rotation (paced by Sign consumption)
            # throttles the queue. Later sets follow behind x2.
            def load_wset(oq):
                wbs = []
                for k in range(KB):
                    wt = wqp.tile([128, 512], dt.bfloat16, name="wt",
                                  tag="wt")
                    nc.sync.dma_start(
                        wt[:],
                        wb_d[k * 128:(k + 1) * 128, oq * 512:(oq + 1) * 512])
                    wbs.append(wt)
                w8s = []
                for j in range(KJ):
                    w8 = wqp.tile([128, 2, 512], dt.float8e4, name="w8",
                                  tag="w8", bufs=16)
                    nc.sync.dma_start(
                        w8[:],
                        w8_d[j * 128:(j + 1) * 128, :,
                             oq * 512:(oq + 1) * 512])
                    w8s.append(w8)
                return wbs, w8s

            wsets = [load_wset(0), load_wset(1)]

            hb = []
            hp = []
            for j in range(KJ):
                hp.append(hpp.tile([128, 2, BS], dt.float8e4, name=f"hp{j}",
                                   tag=f"hp{j}"))

            def emit_sign(t):
                x2 = x2p.tile([128, BS], dt.float32, name="x2")
                nc.sync.dma_start(x2[:], xt_d[t * 128:(t + 1) * 128, :])
                if t < KB:
                    hbt = hbp.tile([128, BS], dt.float8e4, name=f"hb{t}",
                                   tag=f"hb{t}")
                    dst = hbt[:]
                    hb.append(hbt)
                else:
                    j, plane = divmod(t - KB, 2)
                    dst = hp[j][:, plane, :]
                at, bt_ = coef(t)
                return chain_act(nc.scalar.activation(dst, x2[:], AF.Sign,
                                                      bias=bt_, scale=at))

            # Interleave coefficient chains with the sign blocks: half 1's
            # Sqrt (an ACT op gated on the second AllReduce) must execute
            # BEHIND the half-0 signs on the in-order ACT queue, or it
            # stalls them until the AllReduce lands. The scheduler hoists it
            # otherwise, so pin it with an explicit edge.
            emit_coeffs(0)
            for t in range(HKS[0]):
                emit_sign(t)
            chain_act(emit_coeffs(1))
            for t in range(HKS[0], KT):
                emit_sign(t)

            # ---- phase 3: out[bt, oq] accumulates 16 bf16 + 8 DoubleRow
            # matmuls per [128, 512] tile. W streams once (oq-major). --------
            # beta: |h| = Abs(x*a + b) on ACT from a third x pass (off the
            # critical path), partition-reduced by ones-matmuls on the PE,
            # spliced into the middle of phase 3.
            hpairs = []

            def emit_habs():
                # Third x pass on the gpsimd queue: lands behind the first oq
                # groups' output writes, keeping this 16MB re-read clear of
                # the W prefetch window that feeds the PE. All 16 |h| pair
                # tiles stay resident so the Abs stream never throttles the
                # gpsimd queue (output writes share it).
                for m in range(KT // 2):
                    hpair = habsp.tile([128, 2, BS], dt.float8e4,
                                       name=f"habs{m}", tag=f"habs{m}")
                    for plane in range(2):
                        t = 2 * m + plane
                        x3 = x3p.tile([128, BS], dt.float32, name="x3")
                        nc.gpsimd.dma_start(x3[:],
                                            xt_d[t * 128:(t + 1) * 128, :])
                        at, bt_ = coef(t)
                        chain_act(nc.scalar.activation(hpair[:, plane, :],
                                                       x3[:], AF.Abs,
                                                       bias=bt_, scale=at))
                    hpairs.append(hpair)

            def emit_beta_mms():
                # |h| pairs in fp8 feed DoubleRow ones-matmuls (32 identical
                # output rows; row 0 is drained): half the PE cost of a bf16
                # ones-reduction. e4m3 |h| only perturbs beta by
                # ~3.6%/sqrt(4096).
                beta_ps = psbp.tile([32, BS], dt.float32, tag="beta", bufs=1)
                for m in range(KT // 2):
                    for half in range(BS // 512):
                        nc.tensor.matmul(
                            beta_ps[:, half * 512:(half + 1) * 512],
                            ones8[:],
                            hpairs[m][:, :, half * 512:(half + 1) * 512],
                            start=(m == 0), stop=(m == KT // 2 - 1),
                            perf_mode=PM.DoubleRow)
                bsum_sb = const.tile([1, BS], dt.float32)
                nc.vector.tensor_copy(bsum_sb[:], beta_ps[0:1, :])
                nc.scalar.dma_start(bsum_d[:], bsum_sb[:])

            for oq in range(D // 512):
                if oq >= 2:
                    wsets.append(load_wset(oq))
                wbs, w8s = wsets[oq]

                def emit_mms(pt, bt, js):
                    if js.start == 0:
                        for k in range(KB):
                            nc.tensor.matmul(
                                pt[:], hb[k][:, bt * 128:(bt + 1) * 128],
                                wbs[k][:], start=(k == 0), stop=False)
                    for j in range(js.start, js.stop):
                        nc.tensor.matmul(
                            pt[:], hp[j][:, :, bt * 128:(bt + 1) * 128],
                            w8s[j][:], start=False, stop=(j == KJ - 1),
                            perf_mode=PM.DoubleRow)

                def drain(pt, bt):
                    yb = ybp.tile([128, 512], dt.bfloat16, name="yb")
                    nc.vector.tensor_copy(yb[:], pt[:])
                    nc.gpsimd.dma_start(
                        out_d[bt * 128:(bt + 1) * 128,
                              oq * 512:(oq + 1) * 512], yb[:])

                if oq == 0:
                    # Bridge the AllReduce window: six passes advance k-major
                    # (each new sign tile feeds six matmuls) through the
                    # half-0-covered range, then finish once the half-1 signs
                    # land.
                    pts = [psp.tile([128, 512], dt.float32, name="pt",
                                    tag="pt") for _ in range(6)]
                    for k in range(KB):
                        for bt in range(6):
                            nc.tensor.matmul(
                                pts[bt][:], hb[k][:, bt * 128:(bt + 1) * 128],
                                wbs[k][:], start=(k == 0), stop=False)
                    for j in range(4):
                        for bt in range(6):
                            nc.tensor.matmul(
                                pts[bt][:],
                                hp[j][:, :, bt * 128:(bt + 1) * 128],
                                w8s[j][:], start=False, stop=False,
                                perf_mode=PM.DoubleRow)
                    for bt in range(6):
                        emit_mms(pts[bt], bt, slice(4, KJ))
                        drain(pts[bt], bt)
                    rest = range(6, BS // 128)
                else:
                    rest = range(BS // 128)
                for bt in rest:
                    pt = psp.tile([128, 512], dt.float32, name="pt", tag="pt")
                    emit_mms(pt, bt, slice(0, KJ))
                    drain(pt, bt)
                if oq == 2:
                    emit_habs()
                if oq == 5:
                    emit_beta_mms()

    nc.compile()
    return nc


def kernel(x, bn_gamma, bn_beta, W, alpha):
    global _nc_cache, LAST_RESULT
    x = np.ascontiguousarray(x, dtype=np.float32)
    W = np.asarray(W, dtype=np.float32)
    alpha = np.asarray(alpha, dtype=np.float32)

    # host prep: fold alpha and the fp8 range scale into W, transpose to
    # [in, out]; k-rows 0..2047 in bf16, 2048..4095 in fp8 e4m3 arranged as
    # DoubleRow pair planes (k-tile 16+2j -> plane 0, 17+2j -> plane 1).
    wt = np.ascontiguousarray((W * alpha[:, None]).T) * np.float32(WSCALE)
    wb = wt[:KB * 128].astype(ml_dtypes.bfloat16)
    w8flat = wt[KB * 128:].astype(ml_dtypes.float8_e4m3)
    w8 = np.ascontiguousarray(
        w8flat.reshape(KJ, 2, 128, D).swapaxes(1, 2).reshape(KJ * 128, 2, D))
    # gamma/beta in per-partition layout: gb[p, t] = gamma[t*128 + p]
    gb = np.concatenate(
        [np.asarray(bn_gamma, np.float32).reshape(KT, 128).T,
         np.asarray(bn_beta, np.float32).reshape(KT, 128).T], axis=1)
    gb = np.ascontiguousarray(gb)

    if _nc_cache is None:
        _nc_cache = _build()
    nc = _nc_cache

    in_maps = []
    for c in range(N_CORES):
        xT = np.ascontiguousarray(x[c * BS:(c + 1) * BS, :].T)
        in_maps.append({"xt": xT, "wb": wb, "w8": w8, "gb": gb})

    res = run_bass_kernel_spmd(nc, in_maps, core_ids=list(range(N_CORES)),
                               trace=TRACE)
    LAST_RESULT = res
    outs = []
    for c in range(N_CORES):
        raw = res.results[c]["out"].astype(np.float32)        # [BS, D]
        bsum = np.asarray(res.results[c]["bsum"], np.float32)  # [1, BS]
        scale = bsum[0] / np.float32(D * WSCALE)               # [BS]
        outs.append(raw * scale[:, None])
    return np.concatenate(outs, axis=0)


# revision 26
# speedup vs baseline: 1.0353x; 1.0051x over previous
"""BinLinear (BatchNorm -> sign-binarize -> scaled binary linear) on 8 TRN2
NeuronCores.

Reference computation (fp32, full batch):
    mean/var over batch axis of x [8192, 4096]
    h  = (x - mean) * rsqrt(var + eps) * gamma + beta          [8192, 4096]
    hb = sign(h)
    out = (hb @ W.T) * alpha[None, :] * mean_abs(h, axis=1)    [8192, 4096]

Distribution: data-parallel over the batch (1024 rows per core); BN stats via
a 32 KB AllReduce (a tiny warmup AllReduce at t=0 absorbs the collective
bootstrap skew). W is pre-scaled by alpha and 2^16, transposed to [in, out]
on the host, and split by k: the first 16 k-tiles are bf16, the last 16 are
fp8 e4m3 run as true DoubleRow pairs (two k-slices per PE pass, 2x the bf16
matmul rate). The measured end-to-end rel err of this split on the reference
inputs is 1.889e-2 (fp8-only would be 2.65e-2, all-bf16 ~2e-3), deterministic
across runs.

The kernel returns the raw accumulator (bf16) and the per-row sum of |h|;
the per-row beta scale, the 2^-16 descale and the fp32 cast happen on the
host while gathering shards.

PE timeline per core: 64 output tiles x (16 bf16 + 8 DoubleRow) matmuls
= 786k PE cycles (vs 1049k all-bf16), with stationary loads hidden behind
the moving stream (verified by microbenchmark).
"""

import numpy as np
import ml_dtypes

import concourse.bacc as bacc
import concourse.mybir as mybir
import concourse.tile as tile
from concourse.tile import add_dep_helper
from concourse.bass_utils import run_bass_kernel_spmd

dt = mybir.dt
AF = mybir.ActivationFunctionType
ALU = mybir.AluOpType
PM = mybir.MatmulPerfMode

N_CORES = 8
B, D = 8192, 4096          # batch, features (D_in == D_out == 4096)
BS = B // N_CORES          # 1024 batch rows per core
KT = D // 128              # 32 k-tiles (features / partitions)
KB = 16                    # k-tiles 0..KB-1 run in bf16
KJ = (KT - KB) // 2        # 8 DoubleRow pair groups for k-tiles KB..KT-1
EPS = 1e-5
WSCALE = 2.0 ** 16         # fp8 range scale, folded into both W halves

TRACE = False              # set by test.py for profiling runs
LAST_RESULT = None

_nc_cache = None


def _build():
    nc = bacc.Bacc("TRN2", target_bir_lowering=False, debug=False,
                   num_devices=N_CORES)
    xt_d = nc.dram_tensor("xt", [D, BS], dt.float32, kind="ExternalInput").ap()
    wb_d = nc.dram_tensor("wb", [KB * 128, D], dt.bfloat16,
                          kind="ExternalInput").ap()
    w8_d = nc.dram_tensor("w8", [KJ * 128, 2, D], dt.float8e4,
                          kind="ExternalInput").ap()
    gb_d = nc.dram_tensor("gb", [128, 2 * KT], dt.float32,
                          kind="ExternalInput").ap()
    out_d = nc.dram_tensor("out", [BS, D], dt.bfloat16,
                           kind="ExternalOutput").ap()
    bsum_d = nc.dram_tensor("bsum", [1, BS], dt.float32,
                            kind="ExternalOutput").ap()

    with tile.TileContext(nc) as tc:
        with (
            tc.tile_pool(name="const", bufs=1) as const,
            tc.tile_pool(name="xs", bufs=4) as xsp,
            tc.tile_pool(name="sq", bufs=2) as sqp,
            tc.tile_pool(name="x2", bufs=8) as x2p,
            tc.tile_pool(name="x3", bufs=3) as x3p,
            tc.tile_pool(name="habs", bufs=1) as habsp,
            tc.tile_pool(name="hb", bufs=1) as hbp,
            tc.tile_pool(name="hp", bufs=1) as hpp,
            tc.tile_pool(name="wq", bufs=32) as wqp,
            tc.tile_pool(name="yb", bufs=4) as ybp,
            tc.tile_pool(name="ps", bufs=6, space="PSUM") as psp,
            tc.tile_pool(name="psb", bufs=1, space="PSUM") as psbp,
            tc.tile_pool(name="dram", bufs=1, space="DRAM") as dram,
        ):
            # ---- warmup collective: absorbs bootstrap barrier + core skew
            # while phase 1 streams x. ----------------------------------------
            warm_sb = const.tile([128, 1], dt.float32)
            nc.vector.memset(warm_sb[:], 1.0)
            warm_in = dram.tile([128, 1], dt.float32, name="warm_in",
                                tag="warm_in")
            warm_out = dram.tile([128, 1], dt.float32, name="warm_out",
                                 tag="warm_out")
            nc.gpsimd.dma_start(warm_in[:], warm_sb[:])
            nc.gpsimd.collective_compute(
                "AllReduce", ALU.add,
                replica_groups=[list(range(N_CORES))],
                ins=[warm_in.opt()], outs=[warm_out.opt()],
            )

            # ---- constants -------------------------------------------------
            # Dummy Sqrt as the FIRST activation: pins the initial ACT
            # function-table load to a set that contains square, sqrt, sign
            # AND abs, so no ~20us table reload lands mid-kernel between the
            # stats squares and the sign stream.
            warmact = const.tile([128, 1], dt.float32)
            nc.scalar.activation(warmact[:], warm_sb[:], AF.Sqrt)

            gb_t = const.tile([128, 2 * KT], dt.float32)
            nc.sync.dma_start(gb_t[:], gb_d[:])
            eps_t = const.tile([128, 1], dt.float32)
            nc.vector.memset(eps_t[:], EPS)
            ones8 = const.tile([128, 2, 32], dt.float8e4)
            nc.vector.memset(ones8[:], 1.0)

            # ---- phase 1: per-shard feature sums ---------------------------
            # Asymmetric split (24/8): the first AllReduce covers all bf16
            # k-tiles plus the first 4 DoubleRow pairs, so the PE can run
            # 20 of each pass's 24 matmuls while the second AllReduce is
            # still in flight. stat columns per half: [sum | sumsq].
            HKS = [24, KT - 24]
            HBASE = [0, 2 * HKS[0]]
            KBASE = [0, HKS[0]]
            stat = const.tile([128, 2 * KT], dt.float32)
            for t in range(KT):
                half = 0 if t < HKS[0] else 1
                idx = t - KBASE[half]
                xs = xsp.tile([128, BS], dt.float32, name="xs")
                nc.sync.dma_start(xs[:], xt_d[t * 128:(t + 1) * 128, :])
                c = HBASE[half] + idx
                nc.vector.reduce_sum(stat[:, c:c + 1], xs[:],
                                     axis=mybir.AxisListType.X)
                sq = sqp.tile([128, BS], dt.float32, name="sq")
                c = HBASE[half] + HKS[half] + idx
                nc.scalar.activation(sq[:], xs[:], AF.Square,
                                     accum_out=stat[:, c:c + 1])

            # ---- two pipelined AllReduces (one per feature half). Both
            # input DMAs are issued before either result is read back, so
            # AR1 starts the moment AR0 leaves the ring (a result read on
            # the same queue would stall AR1's input until AR0 completed).
            a_h = [const.tile([128, HKS[h]], dt.float32, name=f"a{h}",
                              tag=f"a{h}") for h in range(2)]
            b_h = [const.tile([128, HKS[h]], dt.float32, name=f"b{h}",
                              tag=f"b{h}") for h in range(2)]
            salls = []
            for half in range(2):
                HK = HKS[half]
                hs = slice(HBASE[half], HBASE[half] + 2 * HK)
                inb = dram.tile([128, 2 * HK], dt.float32, name=f"inb{half}",
                                tag=f"inb{half}")
                outb = dram.tile([128, 2 * HK], dt.float32,
                                 name=f"outb{half}", tag=f"outb{half}")
                nc.scalar.dma_start(inb[:], stat[:, hs])
                nc.gpsimd.collective_compute(
                    "AllReduce", ALU.add,
                    replica_groups=[list(range(N_CORES))],
                    ins=[inb.opt()], outs=[outb.opt()],
                )
                # Result read on the gpsimd queue, BETWEEN the two
                # collectives for half 0: the scalar queue must stay clear so
                # the Sign stream is not serialized behind an AllReduce wait.
                sall = const.tile([128, 2 * HK], dt.float32,
                                  name=f"sall{half}", tag=f"sall{half}")
                nc.gpsimd.dma_start(sall[:], outb[:])
                salls.append(sall)
            def emit_coeffs(half):
                HK = HKS[half]
                sall = salls[half]

                # BN coefficients: a = gamma/std, b = beta - mean*a
                mean = const.tile([128, HK], dt.float32, name=f"mean{half}",
                                  tag=f"mean{half}")
                chain_dve(nc.vector.tensor_scalar_mul(mean[:], sall[:, 0:HK],
                                                      1.0 / B))
                var = const.tile([128, HK], dt.float32, name=f"var{half}",
                                 tag=f"var{half}")
                chain_dve(nc.vector.tensor_scalar_mul(var[:],
                                                      sall[:, HK:2 * HK],
                                                      1.0 / B))
                msq = const.tile([128, HK], dt.float32, name=f"msq{half}",
                                 tag=f"msq{half}")
                chain_dve(nc.vector.tensor_mul(msq[:], mean[:], mean[:]))
                chain_dve(nc.vector.tensor_sub(var[:], var[:], msq[:]))
                std = const.tile([128, HK], dt.float32, name=f"std{half}",
                                 tag=f"std{half}")
                sqrt_inst = nc.scalar.activation(std[:], var[:], AF.Sqrt,
                                                 bias=eps_t[:, 0:1], scale=1.0)
                ivs = const.tile([128, HK], dt.float32, name=f"ivs{half}",
                                 tag=f"ivs{half}")
                chain_dve(nc.vector.reciprocal(ivs[:], std[:]))
                ks = slice(KBASE[half], KBASE[half] + HK)
                chain_dve(nc.vector.tensor_mul(a_h[half][:], ivs[:],
                                               gb_t[:, ks]))
                chain_dve(nc.vector.tensor_mul(b_h[half][:], mean[:],
                                               a_h[half][:]))
                chain_dve(nc.vector.tensor_sub(
                    b_h[half][:],
                    gb_t[:, KT + KBASE[half]:KT + KBASE[half] + HK],
                    b_h[half][:]))
                return sqrt_inst

            # ---- phase 2: binarize. hb = Sign(x*a + b) fused on ACT from a
            # second fp32 pass over x (sync queue, behind phase 1's reads).
            # bf16-range tiles land in plain fp8 tiles; fp8-range tiles land
            # as DoubleRow pair planes. ---------------------------------------
            def coef(t):
                h = 0 if t < HKS[0] else 1
                i = t - KBASE[h]
                return a_h[h][:, i:i + 1], b_h[h][:, i:i + 1]

            # The tile scheduler's timing model misjudges when AllReduce- and
            # late-DMA-gated ACT ops become ready and hoists them between the
            # Sign stream, where their semaphore waits would stall the
            # in-order ACT queue. Chain every ACT op explicitly in program
            # order instead.
            act_chain = [None]

            def chain_act(inst):
                if act_chain[0] is not None:
                    add_dep_helper(inst.ins, act_chain[0].ins, sync=True,
                                   reason="ACT program order")
                act_chain[0] = inst
                return inst

            dve_chain = [None]

            def chain_dve(inst):
                if dve_chain[0] is not None:
                    add_dep_helper(inst.ins, dve_chain[0].ins, sync=True,
                                   reason="DVE program order")
                dve_chain[0] = inst
                return inst

            # W-set loading: sets 0 and 1 are emitted on the sync queue
            # BEFORE the x2 stream so the first passes' weights are on chip
            # before the x2 pool's buffer rotation (paced by Sign consumption)
            # throttles the queue. Later sets follow behind x2.
            def load_wset(oq):
                wbs = []
                for k in range(KB):
                    wt = wqp.tile([128, 512], dt.bfloat16, name="wt",
                                  tag="wt")
                    nc.sync.dma_start(
                        wt[:],
                        wb_d[k * 128:(k + 1) * 128, oq * 512:(oq + 1) * 512])
                    wbs.append(wt)
                w8s = []
                for j in range(KJ):
                    w8 = wqp.tile([128, 2, 512], dt.float8e4, name="w8",
                                  tag="w8", bufs=16)
                    nc.sync.dma_start(
                        w8[:],
                        w8_d[j * 128:(j + 1) * 128, :,
                             oq * 512:(oq + 1) * 512])
                    w8s.append(w8)
                return wbs, w8s

            wsets = [load_wset(0), load_wset(1)]

            hb = []
            hp = []
            for j in range(KJ):
                hp.append(hpp.tile([128, 2, BS], dt.float8e4, name=f"hp{j}",
                                   tag=f"hp{j}"))

            def emit_sign(t):
                x2 = x2p.tile([128, BS], dt.float32, name="x2")
                nc.sync.dma_start(x2[:], xt_d[t * 128:(t + 1) * 128, :])
                if t < KB:
                    hbt = hbp.tile([128, BS], dt.float8e4, name=f"hb{t}",
                                   tag=f"hb{t}")
                    dst = hbt[:]
                    hb.append(hbt)
                else:
                    j, plane = divmod(t - KB, 2)
                    dst = hp[j][:, plane, :]
                at, bt_ = coef(t)
                return chain_act(nc.scalar.activation(dst, x2[:], AF.Sign,
                                                      bias=bt_, scale=at))

            # Interleave coefficient chains with the sign blocks: half 1's
            # Sqrt (an ACT op gated on the second AllReduce) must execute
            # BEHIND the half-0 signs on the in-order ACT queue, or it
            # stalls them until the AllReduce lands. The scheduler hoists it
            # otherwise, so pin it with an explicit edge.
            emit_coeffs(0)
            for t in range(HKS[0]):
                emit_sign(t)
            chain_act(emit_coeffs(1))
            for t in range(HKS[0], KT):
                emit_sign(t)

            # ---- phase 3: out[bt, oq] accumulates 16 bf16 + 8 DoubleRow
            # matmuls per [128, 512] tile. W streams once (oq-major). --------
            # beta: |h| = Abs(x*a + b) on ACT from a third x pass (off the
            # critical path), partition-reduced by ones-matmuls on the PE,
            # spliced into the middle of phase 3.
            hpairs = []

            def emit_habs():
                # Third x pass on the gpsimd queue: lands behind the first oq
                # groups' output writes, keeping this 16MB re-read clear of
                # the W prefetch window that feeds the PE. All 16 |h| pair
                # tiles stay resident so the Abs stream never throttles the
                # gpsimd queue (output writes share it).
                for m in range(KT // 2):
                    hpair = habsp.tile([128, 2, BS], dt.float8e4,
                                       name=f"habs{m}", tag=f"habs{m}")
                    for plane in range(2):
                        t = 2 * m + plane
                        x3 = x3p.tile([128, BS], dt.float32, name="x3")
                        nc.gpsimd.dma_start(x3[:],
                                            xt_d[t * 128:(t + 1) * 128, :])
                        at, bt_ = coef(t)
                        chain_act(nc.scalar.activation(hpair[:, plane, :],
                                                       x3[:], AF.Abs,
                                                       bias=bt_, scale=at))
                    hpairs.append(hpair)

            def emit_beta_mms():
                # |h| pairs in fp8 feed DoubleRow ones-matmuls (32 identical
                # output rows; row 0 is drained): half the PE cost of a bf16
                # ones-reduction. e4m3 |h| only perturbs beta by
                # ~3.6%/sqrt(4096).
                beta_ps = psbp.tile([32, BS], dt.float32, tag="beta", bufs=1)
                for m in range(KT // 2):
                    for half in range(BS // 512):
                        nc.tensor.matmul(
                            beta_ps[:, half * 512:(half + 1) * 512],
                            ones8[:],
                            hpairs[m][:, :, half * 512:(half + 1) * 512],
                            start=(m == 0), stop=(m == KT // 2 - 1),
                            perf_mode=PM.DoubleRow)
                bsum_sb = const.tile([1, BS], dt.float32)
                nc.vector.tensor_copy(bsum_sb[:], beta_ps[0:1, :])
                nc.scalar.dma_start(bsum_d[:], bsum_sb[:])

            for oq in range(D // 512):
                if oq >= 2:
                    wsets.append(load_wset(oq))
                wbs, w8s = wsets[oq]

                def emit_mms(pt, bt, js):
                    if js.start == 0:
                        for k in range(KB):
                            nc.tensor.matmul(
                                pt[:], hb[k][:, bt * 128:(bt + 1) * 128],
                                wbs[k][:], start=(k == 0), stop=False)
                    for j in range(js.start, js.stop):
                        nc.tensor.matmul(
                            pt[:], hp[j][:, :, bt * 128:(bt + 1) * 128],
                            w8s[j][:], start=False, stop=(j == KJ - 1),
                            perf_mode=PM.DoubleRow)

                def drain(pt, bt):
                    yb = ybp.tile([128, 512], dt.bfloat16, name="yb")
                    nc.vector.tensor_copy(yb[:], pt[:])
                    nc.gpsimd.dma_start(
                        out_d[bt * 128:(bt + 1) * 128,
                              oq * 512:(oq + 1) * 512], yb[:])

                if oq == 0:
                    # Bridge the AllReduce window: six passes advance k-major
                    # (each new sign tile feeds six matmuls) through the
                    # half-0-covered range, then finish once the half-1 signs
                    # land.
                    pts = [psp.tile([128, 512], dt.float32, name="pt",
                                    tag="pt") for _ in range(6)]
                    for k in range(KB):
                        for bt in range(6):
                            nc.tensor.matmul(
                                pts[bt][:], hb[k][:, bt * 128:(bt + 1) * 128],
                                wbs[k][:], start=(k == 0), stop=False)
                    for j in range(4):
                        for bt in range(6):
                            nc.tensor.matmul(
                                pts[bt][:],
                                hp[j][:, :, bt * 128:(bt + 1) * 128],
                                w8s[j][:], start=False, stop=False,
                                perf_mode=PM.DoubleRow)
                    for bt in range(6):
                        emit_mms(pts[bt], bt, slice(4, KJ))
                        drain(pts[bt], bt)
                    rest = range(6, BS // 128)
                else:
                    rest = range(BS // 128)
                for bt in rest:
                    pt = psp.tile([128, 512], dt.float32, name="pt", tag="pt")
                    emit_mms(pt, bt, slice(0, KJ))
                    drain(pt, bt)
                if oq == 2:
                    emit_habs()
                if oq == 5:
                    emit_beta_mms()

    nc.compile()
    return nc


def kernel(x, bn_gamma, bn_beta, W, alpha):
    global _nc_cache, LAST_RESULT
    x = np.ascontiguousarray(x, dtype=np.float32)
    W = np.asarray(W, dtype=np.float32)
    alpha = np.asarray(alpha, dtype=np.float32)

    # host prep: fold alpha and the fp8 range scale into W, transpose to
    # [in, out]; k-rows 0..2047 in bf16, 2048..4095 in fp8 e4m3 arranged as
    # DoubleRow pair planes (k-tile 16+2j -> plane 0, 17+2j -> plane 1).
    wt = np.ascontiguousarray((W * alpha[:, None]).T) * np.float32(WSCALE)
    wb = wt[:KB * 128].astype(ml_dtypes.bfloat16)
    w8flat = wt[KB * 128:].astype(ml_dtypes.float8_e4m3)
    w8 = np.ascontiguousarray(
        w8flat.reshape(KJ, 2, 128, D).swapaxes(1, 2).reshape(KJ * 128, 2, D))
    # gamma/beta in per-partition layout: gb[p, t] = gamma[t*128 + p]
    gb = np.concatenate(
        [np.asarray(bn_gamma, np.float32).reshape(KT, 128).T,
         np.asarray(bn_beta, np.float32).reshape(KT, 128).T], axis=1)
    gb = np.ascontiguousarray(gb)

    if _nc_cache is None:
        _nc_cache = _build()
    nc = _nc_cache

    in_maps = []
    for c in range(N_CORES):
        xT = np.ascontiguousarray(x[c * BS:(c + 1) * BS, :].T)
        in_maps.append({"xt": xT, "wb": wb, "w8": w8, "gb": gb})

    res = run_bass_kernel_spmd(nc, in_maps, core_ids=list(range(N_CORES)),
                               trace=TRACE)
    LAST_RESULT = res
    outs = []
    for c in range(N_CORES):
        raw = res.results[c]["out"].astype(np.float32)        # [BS, D]
        bsum = np.asarray(res.results[c]["bsum"], np.float32)  # [1, BS]
        scale = bsum[0] / np.float32(D * WSCALE)               # [BS]
        outs.append(raw * scale[:, None])
    return np.concatenate(outs, axis=0)
